# revision 11
# baseline (speedup 1.0000x reference)
"""MoE MLP Trainium2 Bass kernel (nn_MoEMLP_55061480735482).

Strategy: data-parallel over tokens. The flattened (B*S)=16384 tokens are
split into 8 shards of 2048 (shard c == batch c). Every core holds all 16
experts' weights and routes its own tokens:

  1. router matmul (fp32)  -> logits [2048,16]
  2. top-2 via DVE max8/max_index, softmax weights
  3. expert one-hot -> PE transpose -> cumsum scan (DVE) = per-expert ranks
  4. pair records (token id, gate weight) scatter-added into a
     capacity-padded bucket table in DRAM via SWDGE dma_scatter_add
  5. per expert: gather records back, indirect-DMA gather x rows,
     f32r matmuls (W1^T/W2^T streamed from HBM), gelu, gate-scale,
     indirect-DMA scatter-add into y
  6. l_aux partial sums (softmax prob sums + expert counts) are tiny
     outputs combined on the host.

Everything data-dependent runs on device; the host only reshapes /
transposes inputs, shards, and sums the 8 cores' tiny l_aux partials.
"""

import numpy as np

import concourse.bass as bass
import concourse.bacc as bacc
import concourse.tile as tile
import concourse.mybir as mybir
import concourse.bass_utils as bass_utils

F32 = mybir.dt.float32
F32R = mybir.dt.float32r
BF16 = mybir.dt.bfloat16
I16 = mybir.dt.int16
I32 = mybir.dt.int32
U32 = mybir.dt.uint32
AL = mybir.AluOpType
AF = mybir.ActivationFunctionType

B, S, H, FD, E, TOPK = 8, 2048, 512, 1024, 16, 2
T = 2048            # tokens per core
C = 384             # per-(core, expert) capacity (measured max 336)
NCORES = 8
NT = T // 128       # 16 token tiles
NPAIR = T * TOPK    # 4096 token-slot pairs
NCOL = NPAIR // 128  # 32 pair columns (col = slot*16 + ttile)
MM_DT = F32R        # MLP matmul dtype
GELU_FN = AF.Gelu_apprx_tanh  # matches jax.nn.gelu(approximate=True)

_CACHE = {}


def build_module():
    nc = bacc.Bacc("TRN2", target_bir_lowering=False, debug=False,
                   num_devices=NCORES)

    def inp(name, shape, dt):
        return nc.dram_tensor(name, shape, dt, kind="ExternalInput")

    x_d = inp("x", [T, H], F32)
    xT_d = inp("xT", [H, T], F32)
    wrT_d = inp("wrT", [H, E], F32)
    w1_d = inp("w1t", [E, H, FD], MM_DT)
    w2_d = inp("w2t", [E, FD, H], MM_DT)
    b1_d = inp("b1r", [E, 128, FD // 128], F32)
    b2_d = inp("b2r", [E, 128, H // 128], F32)
    id_d = inp("idn", [128, 128], F32)
    iota_d = inp("iota16", [128, E], F32)
    rep_d = inp("rep16", [16, 128], F32)
    ones_d = inp("ones128", [128, 1], F32)
    tok_d = inp("tokc", [128, NCOL], F32)
    gidx_d = inp("gidx", [128, E * (C // 16)], I16)

    y_d = nc.dram_tensor("y", [T, H], F32, kind="ExternalOutput")
    bkt_d = nc.dram_tensor("bkt", [E * C, 64], F32, kind="ExternalOutput")
    ps_d = nc.dram_tensor("psum_out", [1, E], F32, kind="ExternalOutput")
    cnt_d = nc.dram_tensor("cnt_out", [16, 1], F32, kind="ExternalOutput")

    with tile.TileContext(nc) as tc:
        with tc.tile_pool(name="const", bufs=1) as cp, \
             tc.tile_pool(name="rl", bufs=1) as rl, \
             tc.tile_pool(name="small", bufs=2) as sp, \
             tc.tile_pool(name="wpool", bufs=2) as wp, \
             tc.tile_pool(name="mlp", bufs=2) as mp, \
             tc.tile_pool(name="ps", bufs=8, space="PSUM") as pp:

            _pn = [0]

            def psum(shape, tag="ps"):
                _pn[0] += 1
                return pp.tile(shape, F32, tag=tag, name=f"pt{_pn[0]}")

            # ---------- constants ----------
            idn = cp.tile([128, 128], F32)
            nc.sync.dma_start(idn[:], id_d[:])
            iota16 = cp.tile([128, E], F32)
            nc.sync.dma_start(iota16[:], iota_d[:])
            rep16 = cp.tile([16, 128], F32)
            nc.sync.dma_start(rep16[:], rep_d[:])
            ones128 = cp.tile([128, 1], F32)
            nc.sync.dma_start(ones128[:], ones_d[:])
            tokc = cp.tile([128, NCOL], F32)
            nc.sync.dma_start(tokc[:], tok_d[:])
            gidx = cp.tile([128, E * (C // 16)], I16)
            nc.sync.dma_start(gidx[:], gidx_d[:])
            wrT = cp.tile([128, 4, E], F32)
            nc.sync.dma_start(wrT[:], wrT_d.rearrange("(hb p) e -> p hb e", p=128))

            # ---------- phase R: router ----------
            logits = rl.tile([128, NT, E], F32)
            mx = rl.tile([128, NT, 8], F32)
            mxi = rl.tile([128, NT, 8], U32)
            p_acc = rl.tile([128, E], F32)
            nc.vector.memset(p_acc[:], 0.0)

            for tt in range(NT):
                xt_t = sp.tile([128, 4, 128], F32, tag="xt")
                nc.sync.dma_start(
                    xt_t[:],
                    xT_d[:, tt * 128:(tt + 1) * 128]
                    .rearrange("(hb p) t -> p hb t", p=128))
                lg_ps = psum([128, E])
                for hb in range(4):
                    nc.tensor.matmul(lg_ps[:], xt_t[:, hb, :], wrT[:, hb, :],
                                     start=(hb == 0), stop=(hb == 3))
                nc.vector.tensor_copy(logits[:, tt, :], lg_ps[:])
                nc.vector.max_with_indices(mx[:, tt, :], mxi[:, tt, :],
                                           logits[:, tt, :])

            # top-2 softmax weights: w1 = 1/(1+e^(v2-v1)), w2 = e^(v2-v1)*w1
            w_tm = rl.tile([128, NCOL], F32)       # [:, 0:16]=slot0, [:,16:32]=slot1
            e_tm = rl.tile([128, NCOL], F32)       # expert ids as fp32
            d_all = rl.tile([128, NT], F32)
            exp_d = rl.tile([128, NT], F32)
            nc.vector.tensor_tensor(out=d_all[:], in0=mx[:, :, 1], in1=mx[:, :, 0],
                                    op=AL.subtract)
            nc.scalar.activation(exp_d[:], d_all[:], AF.Exp)
            den = rl.tile([128, NT], F32)
            nc.vector.tensor_scalar(den[:], exp_d[:], 1.0, scalar2=None, op0=AL.add)
            nc.vector.reciprocal(w_tm[:, 0:NT], den[:])
            nc.vector.tensor_tensor(out=w_tm[:, NT:NCOL], in0=exp_d[:],
                                    in1=w_tm[:, 0:NT], op=AL.mult)
            nc.vector.tensor_copy(e_tm[:, 0:NT], mxi[:, :, 0])
            nc.vector.tensor_copy(e_tm[:, NT:NCOL], mxi[:, :, 1])

            # l_aux probs: softmax over all 16 logits, accumulated over tiles
            negv1 = rl.tile([128, NT], F32)
            nc.vector.tensor_scalar(negv1[:], mx[:, :, 0], -1.0, scalar2=None,
                                    op0=AL.mult)
            probs = rl.tile([128, E], F32, tag="probs")
            sume = rl.tile([128, 1], F32, tag="sume")
            rsum = rl.tile([128, 1], F32, tag="rsum")
            for tt in range(NT):
                nc.scalar.activation(probs[:], logits[:, tt, :], AF.Exp,
                                     bias=negv1[:, tt:tt + 1], scale=1.0)
                nc.vector.tensor_reduce(sume[:], probs[:], axis=mybir.AxisListType.X,
                                        op=AL.add)
                nc.vector.reciprocal(rsum[:], sume[:])
                nc.vector.tensor_scalar(probs[:], probs[:], rsum[:, :1],
                                        scalar2=None, op0=AL.mult)
                nc.vector.tensor_tensor(out=p_acc[:], in0=p_acc[:], in1=probs[:],
                                        op=AL.add)
            pr_ps = psum([1, E])
            nc.tensor.matmul(pr_ps[:], ones128[:], p_acc[:], start=True, stop=True)
            pr_sb = rl.tile([1, E], F32)
            nc.vector.tensor_copy(pr_sb[:], pr_ps[:])
            nc.sync.dma_start(ps_d[:], pr_sb[:])

            # ---------- phase L: ranks + bucket scatter ----------
            ot = rl.tile([16, NPAIR], F32)
            o_col = rl.tile([128, E], F32, tag="ocol")
            for col in range(NCOL):
                nc.vector.tensor_scalar(o_col[:], iota16[:], e_tm[:, col:col + 1],
                                        scalar2=None, op0=AL.is_equal)
                otp = psum([16, 128])
                nc.tensor.transpose(otp[:], o_col[:], idn[:])
                nc.vector.tensor_copy(ot[:, col * 128:(col + 1) * 128], otp[:])

            incl = rl.tile([16, NPAIR], F32)
            nc.vector.tensor_tensor_scan(incl[:], ot[:], ot[:], 0.0,
                                         op0=AL.add, op1=AL.bypass)
            cnt_sb = rl.tile([16, 1], F32)
            nc.vector.tensor_copy(cnt_sb[:], incl[:, NPAIR - 1:NPAIR])
            nc.sync.dma_start(cnt_d[:], cnt_sb[:])
            # rank_excl * onehot, still in [16, NPAIR]
            rksel = rl.tile([16, NPAIR], F32)
            nc.vector.tensor_tensor(out=rksel[:], in0=incl[:], in1=ot[:],
                                    op=AL.subtract)  # exclusive ranks
            nc.vector.tensor_tensor(out=rksel[:], in0=rksel[:], in1=ot[:],
                                    op=AL.mult)

            rank_tm = rl.tile([128, NCOL], F32)
            for col in range(NCOL):
                rkp = psum([128, 16])
                nc.tensor.transpose(rkp[:], rksel[:, col * 128:(col + 1) * 128],
                                    idn[:16, :16])
                nc.vector.tensor_reduce(rank_tm[:, col:col + 1], rkp[:],
                                        axis=mybir.AxisListType.X, op=AL.add)

            pos_tm = rl.tile([128, NCOL], F32)
            nc.vector.scalar_tensor_tensor(out=pos_tm[:], in0=e_tm[:],
                                           scalar=float(C), in1=rank_tm[:],
                                           op0=AL.mult, op1=AL.add)

            # wrapped positions: [128, 32] -> [16, 256] -> replicate [128, 256]
            t1p = psum([32, 128])
            nc.tensor.transpose(t1p[:], pos_tm[:], idn[:])
            a_sb = rl.tile([32, 128], F32)
            nc.vector.tensor_copy(a_sb[:], t1p[:])
            pos16 = rl.tile([16, NCOL, 8], F32)
            for jj in range(8):
                t2p = psum([16, 32])
                nc.tensor.transpose(t2p[:], a_sb[:, jj * 16:(jj + 1) * 16],
                                    idn[:32, :32])
                nc.vector.tensor_copy(pos16[:, :, jj], t2p[:])
            repp = psum([128, NPAIR // 16])
            nc.tensor.matmul(repp[:], rep16[:],
                             pos16[:].rearrange("q a b -> q (a b)"),
                             start=True, stop=True)
            pos_w = rl.tile([128, NPAIR // 16], I16)
            nc.vector.tensor_copy(pos_w[:], repp[:])

            # records [128, 32, 64]: col0 = token id, col1 = gate weight
            rec_in = rl.tile([128, NCOL, 64], F32)
            nc.vector.memset(rec_in[:], 0.0)
            # zero the bucket table on-device using the zeroed records tile
            # as source (robust against non-zeroed output buffers; required
            # when timing reruns the NEFF). The later writes into rec_in
            # are WAR-ordered after these reads by Tile.
            bkt_flat = bkt_d.rearrange("(p a) d -> p (a d)", p=128)
            rec_flat = rec_in[:].rearrange("p a d -> p (a d)")
            nc.sync.dma_start(bkt_flat[:, 0:2048], rec_flat[:, 0:2048])
            nc.sync.dma_start(bkt_flat[:, 2048:3072], rec_flat[:, 0:1024])
            nc.vector.tensor_copy(rec_in[:, :, 0], tokc[:])
            nc.vector.tensor_copy(rec_in[:, :, 1], w_tm[:])
            nc.gpsimd.dma_scatter_add(bkt_d[:], rec_in[:], pos_w[:],
                                      num_idxs=NPAIR, num_idxs_reg=NPAIR,
                                      elem_size=64)

            # ---------- phase M: per-expert MLP ----------
            NCH = C // 128  # 3 chunks
            for e in range(E):
                rec = mp.tile([128, NCH, 64], F32, tag="rec")
                nc.gpsimd.dma_gather(rec[:], bkt_d[:],
                                     gidx[:, e * (C // 16):(e + 1) * (C // 16)],
                                     num_idxs=C, num_idxs_reg=C, elem_size=64)
                toki = mp.tile([128, NCH], I32, tag="toki")
                nc.vector.tensor_copy(toki[:], rec[:, :, 0])

                xg = mp.tile([128, NCH, H], F32, tag="xg")
                for c in range(NCH):
                    nc.gpsimd.indirect_dma_start(
                        out=xg[:, c, :], out_offset=None, in_=x_d[:],
                        in_offset=bass.IndirectOffsetOnAxis(
                            ap=toki[:, c:c + 1], axis=0))

                w1sb = wp.tile([128, 4, FD], MM_DT, tag="w1")
                nc.sync.dma_start(
                    w1sb[:], w1_d[e].rearrange("(hb p) f -> p hb f", p=128))
                w2sb = wp.tile([128, 8, H], MM_DT, tag="w2")
                nc.sync.dma_start(
                    w2sb[:], w2_d[e].rearrange("(fb p) h -> p fb h", p=128))
                b1sb = wp.tile([128, FD // 128], F32, tag="b1")
                nc.sync.dma_start(b1sb[:], b1_d[e])
                b2sb = wp.tile([128, H // 128], F32, tag="b2")
                nc.sync.dma_start(b2sb[:], b2_d[e])

                xgt = mp.tile([128, 4, C], MM_DT, tag="xgt")
                for c in range(NCH):
                    for hb in range(4):
                        tp = psum([128, 128])
                        nc.tensor.transpose(
                            tp[:], xg[:, c, hb * 128:(hb + 1) * 128], idn[:])
                        nc.vector.tensor_copy(
                            xgt[:, hb, c * 128:(c + 1) * 128], tp[:])

                h1 = mp.tile([128, FD // 128, C], MM_DT, tag="h1")
                for fb in range(FD // 128):
                    m1 = psum([128, C])
                    for hb in range(4):
                        nc.tensor.matmul(
                            m1[:], w1sb[:, hb, fb * 128:(fb + 1) * 128],
                            xgt[:, hb, :], start=(hb == 0), stop=(hb == 3))
                    nc.scalar.activation(h1[:, fb, :], m1[:], GELU_FN,
                                         bias=b1sb[:, fb:fb + 1], scale=1.0)

                yt = mp.tile([128, H // 128, C], F32, tag="yt")
                for hb2 in range(H // 128):
                    m2 = psum([128, C])
                    for fb in range(FD // 128):
                        nc.tensor.matmul(
                            m2[:], w2sb[:, fb, hb2 * 128:(hb2 + 1) * 128],
                            h1[:, fb, :], start=(fb == 0), stop=(fb == 7))
                    nc.vector.tensor_scalar(yt[:, hb2, :], m2[:],
                                            b2sb[:, hb2:hb2 + 1],
                                            scalar2=None, op0=AL.add)

                yg = mp.tile([128, NCH, H], F32, tag="yg")
                for c in range(NCH):
                    for hb2 in range(H // 128):
                        tb = psum([128, 128])
                        nc.tensor.transpose(
                            tb[:], yt[:, hb2, c * 128:(c + 1) * 128], idn[:])
                        nc.vector.tensor_scalar(
                            yg[:, c, hb2 * 128:(hb2 + 1) * 128], tb[:],
                            rec[:, c, 1:2], scalar2=None, op0=AL.mult)
                for c in range(NCH):
                    nc.gpsimd.indirect_dma_start(
                        out=y_d[:],
                        out_offset=bass.IndirectOffsetOnAxis(
                            ap=toki[:, c:c + 1], axis=0),
                        in_=yg[:, c, :], in_offset=None,
                        compute_op=AL.add)

    nc.compile()
    return nc


def host_inputs(x, Wr, W1, b1, W2, b2):
    """Per-core input dicts (host-side shard + layout prep only)."""
    x = np.asarray(x, np.float32).reshape(B * S, H)
    Wr = np.asarray(Wr, np.float32)
    W1 = np.asarray(W1, np.float32)
    b1 = np.asarray(b1, np.float32)
    W2 = np.asarray(W2, np.float32)
    b2 = np.asarray(b2, np.float32)

    wdt = mybir.dt.np(MM_DT)
    w1t = np.ascontiguousarray(W1.transpose(0, 2, 1)).astype(wdt)  # [E, H, FD]
    w2t = np.ascontiguousarray(W2.transpose(0, 2, 1)).astype(wdt)  # [E, FD, H]
    b1r = np.ascontiguousarray(
        b1.reshape(E, FD // 128, 128).transpose(0, 2, 1))        # [E, 128, 8]
    b2r = np.ascontiguousarray(
        b2.reshape(E, H // 128, 128).transpose(0, 2, 1))         # [E, 128, 4]
    wrT = np.ascontiguousarray(Wr.T)                             # [H, E]

    idn = np.eye(128, dtype=np.float32)
    iota16 = np.tile(np.arange(E, dtype=np.float32), (128, 1))
    rep16 = np.zeros((16, 128), np.float32)
    for p in range(128):
        rep16[p % 16, p] = 1.0
    ones128 = np.ones((128, 1), np.float32)
    # token id per (partition, col): col = slot*16 + ttile; tok = ttile*128+p
    tokc = np.empty((128, NCOL), np.float32)
    for col in range(NCOL):
        tokc[:, col] = (col % NT) * 128 + np.arange(128)
    gidx = np.empty((128, E * (C // 16)), np.int16)
    for e in range(E):
        w0 = np.empty((16, C // 16), np.int16)
        for j in range(C):
            w0[j % 16, j // 16] = e * C + j
        gidx[:, e * (C // 16):(e + 1) * (C // 16)] = np.tile(w0, (8, 1))

    shared = dict(wrT=wrT, w1t=w1t, w2t=w2t, b1r=b1r, b2r=b2r, idn=idn,
                  iota16=iota16, rep16=rep16, ones128=ones128, tokc=tokc,
                  gidx=gidx)
    in_maps = []
    for c in range(NCORES):
        xc = np.ascontiguousarray(x[c * T:(c + 1) * T])
        m = dict(shared)
        m["x"] = xc
        m["xT"] = np.ascontiguousarray(xc.T)
        in_maps.append(m)
    return in_maps


LAST_RESULTS = None


def time_kernel(in_maps, iters=20, warmup=3):
    """Measure per-execution wall time via async-pipelined jax dispatch.

    Returns seconds per iteration (min over batches). Output values are
    garbage (no donated zero buffers) — timing only.
    """
    import time
    import jax
    import jax.numpy as jnp
    from jax.sharding import Mesh, PartitionSpec
    from jax.experimental.shard_map import shard_map
    import concourse.bass2jax as bass2jax
    import concourse.mybir as mb

    nc = _CACHE["nc"]
    bass2jax.install_neuronx_cc_hook()
    in_names, out_names, out_avals = [], [], []
    for alloc in nc.m.functions[0].allocations:
        if not isinstance(alloc, mb.MemoryLocationSet):
            continue
        name = alloc.memorylocations[0].name
        if alloc.kind == "ExternalInput":
            in_names.append(name)
        elif alloc.kind == "ExternalOutput":
            out_names.append(name)
            out_avals.append(jax.core.ShapedArray(
                tuple(alloc.tensor_shape), mb.dt.np(alloc.dtype)))
    pname = nc.partition_id_tensor.name if nc.partition_id_tensor else None
    if pname in in_names:
        in_names.remove(pname)
    n_params = len(in_names)
    all_in = in_names + out_names + ([pname] if pname else [])

    def _body(*args):
        operands = list(args)
        if pname:
            operands.append(bass2jax.partition_id_tensor())
        outs = bass2jax._bass_exec_p.bind(
            *operands, out_avals=tuple(out_avals), in_names=tuple(all_in),
            out_names=tuple(out_names), lowering_input_output_aliases=(),
            sim_require_finite=True, sim_require_nnan=True, nc=nc)
        return tuple(outs)

    devices = jax.devices()[:NCORES]
    mesh = Mesh(np.asarray(devices), ("core",))
    nspec = n_params + len(out_names)
    sharded = jax.jit(shard_map(
        _body, mesh=mesh, in_specs=(PartitionSpec("core"),) * nspec,
        out_specs=(PartitionSpec("core"),) * len(out_names), check_rep=False),
        keep_unused=True)
    concat_in = [np.concatenate([np.asarray(in_maps[c][n])
                                 for c in range(NCORES)], axis=0)
                 for n in in_names]
    concat_zero = [np.zeros((NCORES * a.shape[0], *a.shape[1:]), a.dtype)
                   for a in out_avals]
    args = [jax.device_put(a) for a in concat_in + concat_zero]
    for _ in range(warmup):
        outs = sharded(*args)
    jax.block_until_ready(outs)
    best = float("inf")
    for _ in range(3):
        t0 = time.perf_counter()
        for _ in range(iters):
            outs = sharded(*args)
        jax.block_until_ready(outs)
        best = min(best, (time.perf_counter() - t0) / iters)
    return best


def kernel(x, Wr, W1, b1, W2, b2, k):
    global LAST_RESULTS
    assert int(k) == TOPK
    if "nc" not in _CACHE:
        _CACHE["nc"] = build_module()
    nc = _CACHE["nc"]
    in_maps = host_inputs(x, Wr, W1, b1, W2, b2)
    import os
    res = bass_utils.run_bass_kernel_spmd(
        nc, in_maps, core_ids=list(range(NCORES)))
    LAST_RESULTS = res
    y = np.stack([res.results[c]["y"] for c in range(NCORES)], axis=0)
    y = y.reshape(B, S, H)
    p_tot = sum(res.results[c]["psum_out"][0] for c in range(NCORES))
    c_tot = sum(res.results[c]["cnt_out"][:, 0] for c in range(NCORES))
    P = p_tot / (B * S)
    f = c_tot / (B * S * TOPK)
    l_aux = np.float32(E * np.sum(f * P))
    return y, l_aux


# revision 13
# speedup vs baseline: 6125.2770x; 6125.2770x over previous
"""MoE MLP Trainium2 Bass kernel (nn_MoEMLP_55061480735482).

Strategy: data-parallel over tokens. The flattened (B*S)=16384 tokens are
split into 8 shards of 2048 (shard c == batch c). Every core holds all 16
experts' weights and routes its own tokens:

  1. router matmul (fp32)  -> logits [2048,16]
  2. top-2 via DVE max8/max_index, softmax weights
  3. expert one-hot -> PE transpose -> cumsum scan (DVE) = per-expert ranks
  4. pair records (token id, gate weight) scatter-added into a
     capacity-padded bucket table in DRAM via SWDGE dma_scatter_add
  5. per expert: gather records back, indirect-DMA gather x rows,
     f32r matmuls (W1^T/W2^T streamed from HBM), gelu, gate-scale,
     indirect-DMA scatter-add into y
  6. l_aux partial sums (softmax prob sums + expert counts) are tiny
     outputs combined on the host.

Everything data-dependent runs on device; the host only reshapes /
transposes inputs, shards, and sums the 8 cores' tiny l_aux partials.
"""

import numpy as np

import concourse.bass as bass
import concourse.bacc as bacc
import concourse.tile as tile
import concourse.mybir as mybir
import concourse.bass_utils as bass_utils

F32 = mybir.dt.float32
F32R = mybir.dt.float32r
BF16 = mybir.dt.bfloat16
I16 = mybir.dt.int16
I32 = mybir.dt.int32
U32 = mybir.dt.uint32
AL = mybir.AluOpType
AF = mybir.ActivationFunctionType

B, S, H, FD, E, TOPK = 8, 2048, 512, 1024, 16, 2
T = 2048            # tokens per core
C = 384             # per-(core, expert) capacity (measured max 336)
NCORES = 8
NT = T // 128       # 16 token tiles
NPAIR = T * TOPK    # 4096 token-slot pairs
NCOL = NPAIR // 128  # 32 pair columns (col = slot*16 + ttile)
MM_DT = F32R        # MLP matmul dtype
GELU_FN = AF.Gelu_apprx_tanh  # matches jax.nn.gelu(approximate=True)

_CACHE = {}


def build_module():
    nc = bacc.Bacc("TRN2", target_bir_lowering=False, debug=False,
                   num_devices=NCORES)

    def inp(name, shape, dt):
        return nc.dram_tensor(name, shape, dt, kind="ExternalInput")

    x_d = inp("x", [T, H], F32)
    xT_d = inp("xT", [H, T], F32)
    wrT_d = inp("wrT", [H, E], F32)
    w1_d = inp("w1t", [E, H, FD], MM_DT)
    w2_d = inp("w2t", [E, FD, H], MM_DT)
    b1_d = inp("b1r", [E, 128, FD // 128], F32)
    b2_d = inp("b2r", [E, 128, H // 128], F32)
    id_d = inp("idn", [128, 128], F32)
    iota_d = inp("iota16", [128, E], F32)
    rep_d = inp("rep16", [16, 128], F32)
    ones_d = inp("ones128", [128, 1], F32)
    tok_d = inp("tokc", [128, NCOL], F32)
    gidx_d = inp("gidx", [128, E * (C // 16)], I16)

    y_d = nc.dram_tensor("y", [T, H], F32, kind="ExternalOutput")
    bkt_d = nc.dram_tensor("bkt", [E * C, 64], F32, kind="Internal")
    ps_d = nc.dram_tensor("psum_out", [1, E], F32, kind="ExternalOutput")
    cnt_d = nc.dram_tensor("cnt_out", [16, 1], F32, kind="ExternalOutput")

    with tile.TileContext(nc) as tc:
        with tc.tile_pool(name="const", bufs=1) as cp, \
             tc.tile_pool(name="rl", bufs=1) as rl, \
             tc.tile_pool(name="small", bufs=2) as sp, \
             tc.tile_pool(name="wpool", bufs=2) as wp, \
             tc.tile_pool(name="mlp", bufs=2) as mp, \
             tc.tile_pool(name="ps", bufs=8, space="PSUM") as pp:

            _pn = [0]

            def psum(shape, tag="ps"):
                _pn[0] += 1
                return pp.tile(shape, F32, tag=tag, name=f"pt{_pn[0]}")

            # ---------- constants ----------
            idn = cp.tile([128, 128], F32)
            nc.sync.dma_start(idn[:], id_d[:])
            iota16 = cp.tile([128, E], F32)
            nc.sync.dma_start(iota16[:], iota_d[:])
            rep16 = cp.tile([16, 128], F32)
            nc.sync.dma_start(rep16[:], rep_d[:])
            ones128 = cp.tile([128, 1], F32)
            nc.sync.dma_start(ones128[:], ones_d[:])
            tokc = cp.tile([128, NCOL], F32)
            nc.sync.dma_start(tokc[:], tok_d[:])
            gidx = cp.tile([128, E * (C // 16)], I16)
            nc.sync.dma_start(gidx[:], gidx_d[:])
            wrT = cp.tile([128, 4, E], F32)
            nc.sync.dma_start(wrT[:], wrT_d.rearrange("(hb p) e -> p hb e", p=128))

            # ---------- phase R: router ----------
            logits = rl.tile([128, NT, E], F32)
            mx = rl.tile([128, NT, 8], F32)
            mxi = rl.tile([128, NT, 8], U32)
            p_acc = rl.tile([128, E], F32)
            nc.vector.memset(p_acc[:], 0.0)

            for tt in range(NT):
                xt_t = sp.tile([128, 4, 128], F32, tag="xt")
                nc.sync.dma_start(
                    xt_t[:],
                    xT_d[:, tt * 128:(tt + 1) * 128]
                    .rearrange("(hb p) t -> p hb t", p=128))
                lg_ps = psum([128, E])
                for hb in range(4):
                    nc.tensor.matmul(lg_ps[:], xt_t[:, hb, :], wrT[:, hb, :],
                                     start=(hb == 0), stop=(hb == 3))
                nc.vector.tensor_copy(logits[:, tt, :], lg_ps[:])
                nc.vector.max_with_indices(mx[:, tt, :], mxi[:, tt, :],
                                           logits[:, tt, :])

            # top-2 softmax weights: w1 = 1/(1+e^(v2-v1)), w2 = e^(v2-v1)*w1
            w_tm = rl.tile([128, NCOL], F32)       # [:, 0:16]=slot0, [:,16:32]=slot1
            e_tm = rl.tile([128, NCOL], F32)       # expert ids as fp32
            d_all = rl.tile([128, NT], F32)
            exp_d = rl.tile([128, NT], F32)
            nc.vector.tensor_tensor(out=d_all[:], in0=mx[:, :, 1], in1=mx[:, :, 0],
                                    op=AL.subtract)
            nc.scalar.activation(exp_d[:], d_all[:], AF.Exp)
            den = rl.tile([128, NT], F32)
            nc.vector.tensor_scalar(den[:], exp_d[:], 1.0, scalar2=None, op0=AL.add)
            nc.vector.reciprocal(w_tm[:, 0:NT], den[:])
            nc.vector.tensor_tensor(out=w_tm[:, NT:NCOL], in0=exp_d[:],
                                    in1=w_tm[:, 0:NT], op=AL.mult)
            nc.vector.tensor_copy(e_tm[:, 0:NT], mxi[:, :, 0])
            nc.vector.tensor_copy(e_tm[:, NT:NCOL], mxi[:, :, 1])

            # l_aux probs: softmax over all 16 logits, accumulated over tiles
            negv1 = rl.tile([128, NT], F32)
            nc.vector.tensor_scalar(negv1[:], mx[:, :, 0], -1.0, scalar2=None,
                                    op0=AL.mult)
            probs = rl.tile([128, E], F32, tag="probs")
            sume = rl.tile([128, 1], F32, tag="sume")
            rsum = rl.tile([128, 1], F32, tag="rsum")
            for tt in range(NT):
                nc.scalar.activation(probs[:], logits[:, tt, :], AF.Exp,
                                     bias=negv1[:, tt:tt + 1], scale=1.0)
                nc.vector.tensor_reduce(sume[:], probs[:], axis=mybir.AxisListType.X,
                                        op=AL.add)
                nc.vector.reciprocal(rsum[:], sume[:])
                nc.vector.tensor_scalar(probs[:], probs[:], rsum[:, :1],
                                        scalar2=None, op0=AL.mult)
                nc.vector.tensor_tensor(out=p_acc[:], in0=p_acc[:], in1=probs[:],
                                        op=AL.add)
            pr_ps = psum([1, E])
            nc.tensor.matmul(pr_ps[:], ones128[:], p_acc[:], start=True, stop=True)
            pr_sb = rl.tile([1, E], F32)
            nc.vector.tensor_copy(pr_sb[:], pr_ps[:])
            nc.sync.dma_start(ps_d[:], pr_sb[:])

            # ---------- phase L: ranks + bucket scatter ----------
            ot = rl.tile([16, NPAIR], F32)
            o_col = rl.tile([128, E], F32, tag="ocol")
            for col in range(NCOL):
                nc.vector.tensor_scalar(o_col[:], iota16[:], e_tm[:, col:col + 1],
                                        scalar2=None, op0=AL.is_equal)
                otp = psum([16, 128])
                nc.tensor.transpose(otp[:], o_col[:], idn[:])
                nc.vector.tensor_copy(ot[:, col * 128:(col + 1) * 128], otp[:])

            incl = rl.tile([16, NPAIR], F32)
            nc.vector.tensor_tensor_scan(incl[:], ot[:], ot[:], 0.0,
                                         op0=AL.add, op1=AL.bypass)
            cnt_sb = rl.tile([16, 1], F32)
            nc.vector.tensor_copy(cnt_sb[:], incl[:, NPAIR - 1:NPAIR])
            nc.sync.dma_start(cnt_d[:], cnt_sb[:])
            # rank_excl * onehot, still in [16, NPAIR]
            rksel = rl.tile([16, NPAIR], F32)
            nc.vector.tensor_tensor(out=rksel[:], in0=incl[:], in1=ot[:],
                                    op=AL.subtract)  # exclusive ranks
            nc.vector.tensor_tensor(out=rksel[:], in0=rksel[:], in1=ot[:],
                                    op=AL.mult)

            rank_tm = rl.tile([128, NCOL], F32)
            for col in range(NCOL):
                rkp = psum([128, 16])
                nc.tensor.transpose(rkp[:], rksel[:, col * 128:(col + 1) * 128],
                                    idn[:16, :16])
                nc.vector.tensor_reduce(rank_tm[:, col:col + 1], rkp[:],
                                        axis=mybir.AxisListType.X, op=AL.add)

            pos_tm = rl.tile([128, NCOL], F32)
            nc.vector.scalar_tensor_tensor(out=pos_tm[:], in0=e_tm[:],
                                           scalar=float(C), in1=rank_tm[:],
                                           op0=AL.mult, op1=AL.add)

            # wrapped positions: [128, 32] -> [16, 256] -> replicate [128, 256]
            t1p = psum([32, 128])
            nc.tensor.transpose(t1p[:], pos_tm[:], idn[:])
            a_sb = rl.tile([32, 128], F32)
            nc.vector.tensor_copy(a_sb[:], t1p[:])
            pos16 = rl.tile([16, NCOL, 8], F32)
            for jj in range(8):
                t2p = psum([16, 32])
                nc.tensor.transpose(t2p[:], a_sb[:, jj * 16:(jj + 1) * 16],
                                    idn[:32, :32])
                nc.vector.tensor_copy(pos16[:, :, jj], t2p[:])
            repp = psum([128, NPAIR // 16])
            nc.tensor.matmul(repp[:], rep16[:],
                             pos16[:].rearrange("q a b -> q (a b)"),
                             start=True, stop=True)
            pos_w = rl.tile([128, NPAIR // 16], I16)
            nc.vector.tensor_copy(pos_w[:], repp[:])

            # records [128, 32, 64]: col0 = token id, col1 = gate weight
            rec_in = rl.tile([128, NCOL, 64], F32)
            nc.vector.memset(rec_in[:], 0.0)
            # zero the bucket table on-device using the zeroed records tile
            # as source (robust against non-zeroed output buffers; required
            # when timing reruns the NEFF). The later writes into rec_in
            # are WAR-ordered after these reads by Tile.
            bkt_flat = bkt_d.rearrange("(p a) d -> p (a d)", p=128)
            rec_flat = rec_in[:].rearrange("p a d -> p (a d)")
            nc.sync.dma_start(bkt_flat[:, 0:2048], rec_flat[:, 0:2048])
            nc.sync.dma_start(bkt_flat[:, 2048:3072], rec_flat[:, 0:1024])
            nc.vector.tensor_copy(rec_in[:, :, 0], tokc[:])
            nc.vector.tensor_copy(rec_in[:, :, 1], w_tm[:])
            nc.gpsimd.dma_scatter_add(bkt_d[:], rec_in[:], pos_w[:],
                                      num_idxs=NPAIR, num_idxs_reg=NPAIR,
                                      elem_size=64)

            # ---------- phase M: per-expert MLP ----------
            NCH = C // 128  # 3 chunks
            for e in range(E):
                rec = mp.tile([128, NCH, 64], F32, tag="rec")
                nc.gpsimd.dma_gather(rec[:], bkt_d[:],
                                     gidx[:, e * (C // 16):(e + 1) * (C // 16)],
                                     num_idxs=C, num_idxs_reg=C, elem_size=64)
                toki = mp.tile([128, NCH], I32, tag="toki")
                nc.vector.tensor_copy(toki[:], rec[:, :, 0])

                xg = mp.tile([128, NCH, H], F32, tag="xg")
                for c in range(NCH):
                    nc.gpsimd.indirect_dma_start(
                        out=xg[:, c, :], out_offset=None, in_=x_d[:],
                        in_offset=bass.IndirectOffsetOnAxis(
                            ap=toki[:, c:c + 1], axis=0))

                w1sb = wp.tile([128, 4, FD], MM_DT, tag="w1")
                nc.sync.dma_start(
                    w1sb[:], w1_d[e].rearrange("(hb p) f -> p hb f", p=128))
                w2sb = wp.tile([128, 8, H], MM_DT, tag="w2")
                nc.sync.dma_start(
                    w2sb[:], w2_d[e].rearrange("(fb p) h -> p fb h", p=128))
                b1sb = wp.tile([128, FD // 128], F32, tag="b1")
                nc.sync.dma_start(b1sb[:], b1_d[e])
                b2sb = wp.tile([128, H // 128], F32, tag="b2")
                nc.sync.dma_start(b2sb[:], b2_d[e])

                xgt = mp.tile([128, 4, C], MM_DT, tag="xgt")
                for c in range(NCH):
                    for hb in range(4):
                        tp = psum([128, 128])
                        nc.tensor.transpose(
                            tp[:], xg[:, c, hb * 128:(hb + 1) * 128], idn[:])
                        nc.vector.tensor_copy(
                            xgt[:, hb, c * 128:(c + 1) * 128], tp[:])

                h1 = mp.tile([128, FD // 128, C], MM_DT, tag="h1")
                for fb in range(FD // 128):
                    m1 = psum([128, C])
                    for hb in range(4):
                        nc.tensor.matmul(
                            m1[:], w1sb[:, hb, fb * 128:(fb + 1) * 128],
                            xgt[:, hb, :], start=(hb == 0), stop=(hb == 3))
                    nc.scalar.activation(h1[:, fb, :], m1[:], GELU_FN,
                                         bias=b1sb[:, fb:fb + 1], scale=1.0)

                yt = mp.tile([128, H // 128, C], F32, tag="yt")
                for hb2 in range(H // 128):
                    m2 = psum([128, C])
                    for fb in range(FD // 128):
                        nc.tensor.matmul(
                            m2[:], w2sb[:, fb, hb2 * 128:(hb2 + 1) * 128],
                            h1[:, fb, :], start=(fb == 0), stop=(fb == 7))
                    nc.vector.tensor_scalar(yt[:, hb2, :], m2[:],
                                            b2sb[:, hb2:hb2 + 1],
                                            scalar2=None, op0=AL.add)

                yg = mp.tile([128, NCH, H], F32, tag="yg")
                for c in range(NCH):
                    for hb2 in range(H // 128):
                        tb = psum([128, 128])
                        nc.tensor.transpose(
                            tb[:], yt[:, hb2, c * 128:(c + 1) * 128], idn[:])
                        nc.vector.tensor_scalar(
                            yg[:, c, hb2 * 128:(hb2 + 1) * 128], tb[:],
                            rec[:, c, 1:2], scalar2=None, op0=AL.mult)
                for c in range(NCH):
                    nc.gpsimd.indirect_dma_start(
                        out=y_d[:],
                        out_offset=bass.IndirectOffsetOnAxis(
                            ap=toki[:, c:c + 1], axis=0),
                        in_=yg[:, c, :], in_offset=None,
                        compute_op=AL.add)

    nc.compile()
    return nc


def host_inputs(x, Wr, W1, b1, W2, b2):
    """Per-core input dicts (host-side shard + layout prep only)."""
    x = np.asarray(x, np.float32).reshape(B * S, H)
    Wr = np.asarray(Wr, np.float32)
    W1 = np.asarray(W1, np.float32)
    b1 = np.asarray(b1, np.float32)
    W2 = np.asarray(W2, np.float32)
    b2 = np.asarray(b2, np.float32)

    wdt = mybir.dt.np(MM_DT)
    w1t = np.ascontiguousarray(W1.transpose(0, 2, 1)).astype(wdt)  # [E, H, FD]
    w2t = np.ascontiguousarray(W2.transpose(0, 2, 1)).astype(wdt)  # [E, FD, H]
    b1r = np.ascontiguousarray(
        b1.reshape(E, FD // 128, 128).transpose(0, 2, 1))        # [E, 128, 8]
    b2r = np.ascontiguousarray(
        b2.reshape(E, H // 128, 128).transpose(0, 2, 1))         # [E, 128, 4]
    wrT = np.ascontiguousarray(Wr.T)                             # [H, E]

    idn = np.eye(128, dtype=np.float32)
    iota16 = np.tile(np.arange(E, dtype=np.float32), (128, 1))
    rep16 = np.zeros((16, 128), np.float32)
    for p in range(128):
        rep16[p % 16, p] = 1.0
    ones128 = np.ones((128, 1), np.float32)
    # token id per (partition, col): col = slot*16 + ttile; tok = ttile*128+p
    tokc = np.empty((128, NCOL), np.float32)
    for col in range(NCOL):
        tokc[:, col] = (col % NT) * 128 + np.arange(128)
    gidx = np.empty((128, E * (C // 16)), np.int16)
    for e in range(E):
        w0 = np.empty((16, C // 16), np.int16)
        for j in range(C):
            w0[j % 16, j // 16] = e * C + j
        gidx[:, e * (C // 16):(e + 1) * (C // 16)] = np.tile(w0, (8, 1))

    shared = dict(wrT=wrT, w1t=w1t, w2t=w2t, b1r=b1r, b2r=b2r, idn=idn,
                  iota16=iota16, rep16=rep16, ones128=ones128, tokc=tokc,
                  gidx=gidx)
    in_maps = []
    for c in range(NCORES):
        xc = np.ascontiguousarray(x[c * T:(c + 1) * T])
        m = dict(shared)
        m["x"] = xc
        m["xT"] = np.ascontiguousarray(xc.T)
        in_maps.append(m)
    return in_maps


LAST_RESULTS = None


def build_dummy_module():
    """IO-identical twin with a trivial body, for timing calibration."""
    nc = bacc.Bacc("TRN2", target_bir_lowering=False, debug=False,
                   num_devices=NCORES)

    def inp(name, shape, dt):
        return nc.dram_tensor(name, shape, dt, kind="ExternalInput")

    inp("x", [T, H], F32); inp("xT", [H, T], F32); inp("wrT", [H, E], F32)
    inp("w1t", [E, H, FD], MM_DT); inp("w2t", [E, FD, H], MM_DT)
    inp("b1r", [E, 128, FD // 128], F32); inp("b2r", [E, 128, H // 128], F32)
    id_d = inp("idn", [128, 128], F32)
    inp("iota16", [128, E], F32); inp("rep16", [16, 128], F32)
    inp("ones128", [128, 1], F32); inp("tokc", [128, NCOL], F32)
    inp("gidx", [128, E * (C // 16)], I16)
    y_d = nc.dram_tensor("y", [T, H], F32, kind="ExternalOutput")
    ps_d = nc.dram_tensor("psum_out", [1, E], F32, kind="ExternalOutput")
    cnt_d = nc.dram_tensor("cnt_out", [16, 1], F32, kind="ExternalOutput")
    with tile.TileContext(nc) as tc:
        with tc.tile_pool(name="sb", bufs=1) as sb:
            t1 = sb.tile([128, 128], F32)
            nc.sync.dma_start(t1[:], id_d[:])
            nc.sync.dma_start(y_d[0:128, 0:128], t1[:])
            t2 = sb.tile([16, E], F32)
            nc.vector.memset(t2[:], 0.0)
            nc.sync.dma_start(ps_d[:], t2[0:1, :])
            nc.sync.dma_start(cnt_d[:], t2[:, 0:1])
    nc.compile()
    return nc


def time_kernel(in_maps, iters=20, warmup=3, nc=None):
    """Measure per-execution wall time via async-pipelined jax dispatch.

    Returns seconds per iteration (min over batches). Output values are
    garbage (no donated zero buffers) — timing only.
    """
    import time
    import jax
    import jax.numpy as jnp
    from jax.sharding import Mesh, PartitionSpec
    from jax.experimental.shard_map import shard_map
    import concourse.bass2jax as bass2jax
    import concourse.mybir as mb

    if nc is None:
        nc = _CACHE["nc"]
    bass2jax.install_neuronx_cc_hook()
    in_names, out_names, out_avals = [], [], []
    for alloc in nc.m.functions[0].allocations:
        if not isinstance(alloc, mb.MemoryLocationSet):
            continue
        name = alloc.memorylocations[0].name
        if alloc.kind == "ExternalInput":
            in_names.append(name)
        elif alloc.kind == "ExternalOutput":
            out_names.append(name)
            out_avals.append(jax.core.ShapedArray(
                tuple(alloc.tensor_shape), mb.dt.np(alloc.dtype)))
    pname = nc.partition_id_tensor.name if nc.partition_id_tensor else None
    if pname in in_names:
        in_names.remove(pname)
    n_params = len(in_names)
    all_in = in_names + out_names + ([pname] if pname else [])

    def _body(*args):
        operands = list(args)
        if pname:
            operands.append(bass2jax.partition_id_tensor())
        outs = bass2jax._bass_exec_p.bind(
            *operands, out_avals=tuple(out_avals), in_names=tuple(all_in),
            out_names=tuple(out_names), lowering_input_output_aliases=(),
            sim_require_finite=True, sim_require_nnan=True, nc=nc)
        return tuple(outs)

    devices = jax.devices()[:NCORES]
    mesh = Mesh(np.asarray(devices), ("core",))
    nspec = n_params + len(out_names)
    sharded = jax.jit(shard_map(
        _body, mesh=mesh, in_specs=(PartitionSpec("core"),) * nspec,
        out_specs=(PartitionSpec("core"),) * len(out_names), check_rep=False),
        keep_unused=True)
    concat_in = [np.concatenate([np.asarray(in_maps[c][n])
                                 for c in range(NCORES)], axis=0)
                 for n in in_names]
    concat_zero = [np.zeros((NCORES * a.shape[0], *a.shape[1:]), a.dtype)
                   for a in out_avals]
    from jax.sharding import NamedSharding
    shd = NamedSharding(mesh, PartitionSpec("core"))
    args = [jax.device_put(a, shd) for a in concat_in + concat_zero]
    for _ in range(warmup):
        outs = sharded(*args)
    jax.block_until_ready(outs)
    best = float("inf")
    for _ in range(3):
        t0 = time.perf_counter()
        for _ in range(iters):
            outs = sharded(*args)
        jax.block_until_ready(outs)
        best = min(best, (time.perf_counter() - t0) / iters)
    return best


def kernel(x, Wr, W1, b1, W2, b2, k):
    global LAST_RESULTS
    assert int(k) == TOPK
    if "nc" not in _CACHE:
        _CACHE["nc"] = build_module()
    nc = _CACHE["nc"]
    in_maps = host_inputs(x, Wr, W1, b1, W2, b2)
    import os
    res = bass_utils.run_bass_kernel_spmd(
        nc, in_maps, core_ids=list(range(NCORES)))
    LAST_RESULTS = res
    y = np.stack([res.results[c]["y"] for c in range(NCORES)], axis=0)
    y = y.reshape(B, S, H)
    p_tot = sum(res.results[c]["psum_out"][0] for c in range(NCORES))
    c_tot = sum(res.results[c]["cnt_out"][:, 0] for c in range(NCORES))
    P = p_tot / (B * S)
    f = c_tot / (B * S * TOPK)
    l_aux = np.float32(E * np.sum(f * P))
    return y, l_aux
